# revision 1
# baseline (speedup 1.0000x reference)
"""DeformConv3D on 8 TRN2 cores: H-sharded, dense 5-tap tent-weight gather.

Per core (h-band of 12 output rows + halos):
  P1: offset conv (27 taps, K=64 matmuls accumulated in PSUM) -> off scratch DRAM
  P2: trilinear gather as separable 5-tap tent-weighted sums on DVE
      (one (b,c) plane per partition; all shifts are AP offsets into a
       padded per-plane window; tent weights vanish outside the clamp range
       so padded reads are weight-zero)
  P3: main conv + bias -> output h-band
"""
import sys, os
import numpy as np
from contextlib import ExitStack

sys.path.insert(0, "/opt/trn_rl_repo")
from concourse import bass, bacc, tile, mybir
from concourse.bass_utils import run_bass_kernel_spmd

F32 = mybir.dt.float32
BF16 = mybir.dt.bfloat16
ALU = mybir.AluOpType
AF = mybir.ActivationFunctionType

B, C, L, H, W = 2, 64, 16, 96, 96
CO1, CO2 = 192, 64
NCORES = 8
HB = H // NCORES       # 12 output rows per core
HW_ROWS = 20           # x window rows per core: [12k-4, 12k+16)
HG = 14                # gather rows per core: [12k-1, 12k+13)
NPP = HG * W           # 1344 gather outputs per (plane, l)
ZPAD, XPAD = 20, 100   # gather window padded dims (taps +-2)
WIN = HW_ROWS * ZPAD * XPAD
CZP, CXP = 18, 98      # conv window padded dims (taps +-1)
TAPS = (-2, -1, 0, 1, 2)

_nc1_cache = None
_nc2_cache = None


def build_program1():
    nc = bacc.Bacc("TRN2", target_bir_lowering=False, debug=False, num_devices=NCORES)
    xwin = nc.dram_tensor("xwin", [B, C, L, HW_ROWS, W], F32, kind="ExternalInput").ap()
    w_off = nc.dram_tensor("w_off", [27, C, CO1], F32, kind="ExternalInput").ap()
    off_scr = nc.dram_tensor("off_band", [B, CO1, L, HG, W], F32, kind="ExternalOutput").ap()
    ctx = ExitStack()
    with tile.TileContext(nc) as tc:
        # ---------------- Phase 1: offset conv ----------------
        with tc.tile_pool(name="p1", bufs=1) as p1, \
             tc.tile_pool(name="p1ps", bufs=2, space="PSUM") as p1ps, \
             tc.tile_pool(name="p1o", bufs=3) as p1o:
            wofft = p1.tile([C, 27, CO1], F32)
            nc.sync.dma_start(wofft[:], w_off.rearrange("t c m -> c t m"))
            for b in range(B):
                xc = p1.tile([C, CZP, HW_ROWS, CXP], F32, tag="xc")
                nc.vector.memset(xc[:].rearrange("c z y x -> c (z y x)"), 0.0)
                for z in range(L):
                    nc.sync.dma_start(xc[:, 1 + z, :, 1:W + 1], xwin[b, :, z])
                for l in range(L):
                    for hc0, hcn in ((0, 5), (5, 5), (10, 4)):
                        nmm = hcn * W
                        for m0, mw in ((0, 128), (128, 64)):
                            ps = p1ps.tile([128, 480], F32, tag="ps1")
                            for t in range(27):
                                dz, rem = divmod(t, 9)
                                dy, dx = divmod(rem, 3)
                                rhs = xc[:, l + dz,
                                         2 + hc0 + dy:2 + hc0 + dy + hcn,
                                         dx:dx + W]
                                nc.tensor.matmul(
                                    ps[:mw, :nmm], wofft[:, t, m0:m0 + mw],
                                    rhs, start=(t == 0), stop=(t == 26))
                            ob = p1o.tile([128, 480], F32, tag="ob1")
                            nc.vector.tensor_copy(ob[:mw, :nmm], ps[:mw, :nmm])
                            nc.sync.dma_start(
                                off_scr[b, m0:m0 + mw, l, hc0:hc0 + hcn, :]
                                .rearrange("m h x -> m (h x)"),
                                ob[:mw, :nmm])

    nc.finalize()
    return nc


def build_program2():
    nc = bacc.Bacc("TRN2", target_bir_lowering=False, debug=False, num_devices=NCORES)
    xwin = nc.dram_tensor("xwin", [B, C, L, HW_ROWS, W], BF16, kind="ExternalInput").ap()
    w_conv = nc.dram_tensor("w_conv", [27, C, CO2], F32, kind="ExternalInput").ap()
    b_conv = nc.dram_tensor("b_conv", [CO2, 1], F32, kind="ExternalInput").ap()
    offs = nc.dram_tensor("offs", [128, 3, L, NPP], F32, kind="ExternalInput").ap()
    grids = nc.dram_tensor("grids", [128, 1, NPP], F32, kind="ExternalInput").ap()
    out_ext = nc.dram_tensor("out", [B, CO2, L, HB, W], F32, kind="ExternalOutput").ap()
    def_scr = nc.dram_tensor("def_scr", [B, C, L, HG, W], F32).ap()
    ctx = ExitStack()
    with tile.TileContext(nc) as tc:
        # ---------------- Phase 2: tent gather ----------------
        with tc.tile_pool(name="p2w", bufs=1) as p2w, \
             tc.tile_pool(name="p2", bufs=1) as p2:
            win = p2w.tile([128, HW_ROWS, ZPAD, XPAD], BF16)
            nc.vector.memset(win[:].rearrange("p y z x -> p (y z x)"), 0.0)
            for b in range(B):
                for z in range(L):
                    nc.sync.dma_start(
                        win[64 * b:64 * b + 64, :, 2 + z, 2:W + 2],
                        xwin[b, :, z])
            gr = p2w.tile([128, 1, NPP], F32)
            nc.sync.dma_start(gr[:], grids)
            zbias = p2w.tile([128, 1], F32)
            nc.vector.memset(zbias[:], 0.0)

            for l in range(L):
                offc = p2.tile([128, 3, NPP], F32, tag="off")
                nc.sync.dma_start(offc[:], offs[:, :, l, :])
                az = offc[:, 0]
                ay = offc[:, 1]
                ax = offc[:, 2]

                # tent weights lam[dim][tap] = relu(1 - |a - t|)  (bf16)
                tneg = p2.tile([128, NPP], F32, tag="tneg")
                tpos = p2.tile([128, NPP], F32, tag="tpos")

                def tents(a, dst_tag, taps):
                    row = []
                    for t in taps:
                        nc.vector.tensor_scalar(tpos[:], a, 1.0 - float(t), None, ALU.add)
                        nc.vector.tensor_scalar(tneg[:], a, -1.0, 1.0 + float(t), ALU.mult, ALU.add)
                        nc.vector.tensor_tensor(tpos[:], tpos[:], tneg[:], ALU.min)
                        lt = p2.tile([128, NPP], BF16, tag=f"{dst_tag}_{t}")
                        nc.scalar.activation(lt[:], tpos[:], AF.Relu, bias=zbias[:])
                        row.append(lt)
                    return row

                lamx = tents(ax, "lamx", TAPS)
                lamy = tents(ay, "lamy", TAPS)

                acc = p2.tile([128, NPP], F32, tag="acc")
                tmpi = p2.tile([128, NPP], F32, tag="tmpi")
                tmpb = p2.tile([128, NPP], F32, tag="tmpb")
                prod = p2.tile([128, NPP], BF16, tag="prod")
                lam = [None, lamy, lamx]
                for iz, sz in enumerate(TAPS):
                    lamz = tents(az, "lamz", (sz,))[0]
                    for iy, sy in enumerate(TAPS):
                        for ix, sx in enumerate(TAPS):
                            v = win[:, 3 + sy:3 + sy + HG,
                                    l + 2 + sz,
                                    2 + sx:2 + sx + W]
                            if ix == 0:
                                nc.vector.tensor_tensor(tmpi[:], lam[2][0][:], v, ALU.mult)
                            else:
                                nc.vector.tensor_tensor(prod[:], lam[2][ix][:], v, ALU.mult)
                                nc.vector.tensor_tensor(tmpi[:], tmpi[:], prod[:], ALU.add)
                        if iy == 0:
                            nc.vector.tensor_tensor(tmpb[:], lam[1][0][:], tmpi[:], ALU.mult)
                        else:
                            nc.vector.tensor_tensor(tmpi[:], lam[1][iy][:], tmpi[:], ALU.mult)
                            nc.vector.tensor_tensor(tmpb[:], tmpb[:], tmpi[:], ALU.add)
                    if iz == 0:
                        nc.vector.tensor_tensor(acc[:], lamz[:], tmpb[:], ALU.mult)
                    else:
                        nc.vector.tensor_tensor(tmpb[:], lamz[:], tmpb[:], ALU.mult)
                        nc.vector.tensor_tensor(acc[:], acc[:], tmpb[:], ALU.add)
                # zero rows whose global h is outside [0, 96)
                nc.vector.tensor_tensor(acc[:], acc[:], gr[:, 0], ALU.mult)
                for b in range(B):
                    nc.sync.dma_start(
                        def_scr[b, :, l].rearrange("c h x -> c (h x)"),
                        acc[64 * b:64 * b + 64, :])

        # ---------------- Phase 3: main conv ----------------
        with tc.tile_pool(name="p3", bufs=1) as p3, \
             tc.tile_pool(name="p3ps", bufs=2, space="PSUM") as p3ps, \
             tc.tile_pool(name="p3o", bufs=3) as p3o:
            wct = p3.tile([C, 27, CO2], F32)
            nc.sync.dma_start(wct[:], w_conv.rearrange("t c m -> c t m"))
            bct = p3.tile([CO2, 1], F32)
            nc.sync.dma_start(bct[:], b_conv)
            for b in range(B):
                dc = p3.tile([C, CZP, HG + 2, CXP], F32, tag="dc")
                nc.vector.memset(dc[:].rearrange("c z y x -> c (z y x)"), 0.0)
                for z in range(L):
                    nc.sync.dma_start(dc[:, 1 + z, 1:HG + 1, 1:W + 1], def_scr[b, :, z])
                for l in range(L):
                    for hc0, hcn in ((0, 5), (5, 5), (10, 2)):
                        nmm = hcn * W
                        ps = p3ps.tile([CO2, 480], F32, tag="ps3")
                        for t in range(27):
                            dz, rem = divmod(t, 9)
                            dy, dx = divmod(rem, 3)
                            # out row r=4+hc0+j -> dc y index r+dy-3
                            rhs = dc[:, l + dz,
                                     1 + hc0 + dy:1 + hc0 + dy + hcn,
                                     dx:dx + W]
                            nc.tensor.matmul(
                                ps[:, :nmm], wct[:, t, :],
                                rhs, start=(t == 0), stop=(t == 26))
                        ob = p3o.tile([CO2, 480], F32, tag="ob3")
                        nc.vector.tensor_scalar(ob[:, :nmm], ps[:, :nmm], bct[:], None, ALU.add)
                        nc.sync.dma_start(
                            out_ext[b, :, l, hc0:hc0 + hcn, :]
                            .rearrange("m h x -> m (h x)"),
                            ob[:, :nmm])
    nc.finalize()
    return nc


def kernel(x, w_off, w_conv, b_conv):
    global _nc1_cache, _nc2_cache
    x = np.asarray(x, dtype=np.float32)
    w_off = np.asarray(w_off, dtype=np.float32)
    w_conv = np.asarray(w_conv, dtype=np.float32)
    b_conv = np.asarray(b_conv, dtype=np.float32)

    if _nc1_cache is None:
        _nc1_cache = build_program1()
        _nc2_cache = build_program2()

    xp = np.zeros((B, C, L, H + 8, W), np.float32)
    xp[:, :, :, 4:4 + H, :] = x
    wofft = np.ascontiguousarray(
        w_off.reshape(CO1, C, 27).transpose(2, 1, 0))        # [27, C, CO1]
    wct = np.ascontiguousarray(
        w_conv.reshape(CO2, C, 27).transpose(2, 1, 0))       # [27, C, CO2]
    bc = np.ascontiguousarray(b_conv.reshape(CO2, 1))

    xwins = [np.ascontiguousarray(xp[:, :, :, 12 * k:12 * k + HW_ROWS, :])
             for k in range(NCORES)]
    import ml_dtypes
    xwins_bf = [w.astype(ml_dtypes.bfloat16) for w in xwins]
    in1 = [{"xwin": xwins[k], "w_off": wofft} for k in range(NCORES)]
    res1 = run_bass_kernel_spmd(_nc1_cache, in1, list(range(NCORES)))

    # reassemble full off field from per-core bands (band rows = 12k-1..12k+13)
    off_full = np.empty((B, CO1, L, H, W), np.float32)
    for k in range(NCORES):
        band = res1.results[k]["off_band"]
        off_full[:, :, :, 12 * k:12 * k + HB, :] = band[:, :, :, 1:1 + HB, :]
    # contiguous-view scramble: plane (b,c) offsets at spatial p, comp k =
    # flat element 3p+k of its 3-channel block. Per (l, h) row that is a
    # contiguous 288-float run, so a padded reshape + slice does it all.
    tri = off_full.reshape(B * C, L, 3 * H * W)
    trip = np.zeros((B * C, L, 3 * (H + 2) * W), np.float32)
    trip[:, :, 3 * W:3 * (H + 1) * W] = tri            # one pad row each side
    trip = trip.reshape(B * C, L, H + 2, W * 3)
    in2 = []
    gy = np.repeat(np.arange(HG, dtype=np.float32) + 3.0, W)
    gx = np.tile(np.arange(W, dtype=np.float32), HG)
    lgrid = np.arange(L, dtype=np.float32)[None, None, :, None]
    for k in range(NCORES):
        seg = trip[:, :, 12 * k:12 * k + HG, :]        # rows 12k-1..12k+13
        offs = np.ascontiguousarray(
            seg.reshape(128, L, HG * W, 3).transpose(0, 3, 1, 2))
        # displacements a = clamp(off + grid) - grid, computed on host
        hglobf = np.repeat(np.arange(HG, dtype=np.float32) + (12 * k - 1), W)
        offs[:, 0] = np.clip(offs[:, 0] + lgrid[0], 0.0, 15.0) - lgrid[0]
        offs[:, 1] = (np.clip(offs[:, 1] + hglobf[None, None, :], 0.0, 95.0)
                      - hglobf[None, None, :])
        offs[:, 2] = (np.clip(offs[:, 2] + gx[None, None, :], 0.0, 95.0)
                      - gx[None, None, :])
        hglob = np.repeat(np.arange(HG) + (12 * k - 1), W)
        ymask = ((hglob >= 0) & (hglob < H)).astype(np.float32)
        grids = np.broadcast_to(ymask[None, None], (128, 1, NPP)).copy()
        in2.append({
            "xwin": xwins_bf[k], "w_conv": wct, "b_conv": bc,
            "offs": offs,
            "grids": grids,
        })
    res2 = run_bass_kernel_spmd(_nc2_cache, in2, list(range(NCORES)))
    out = np.empty((B, CO2, L, H, W), np.float32)
    for k in range(NCORES):
        out[:, :, :, 12 * k:12 * k + HB, :] = res2.results[k]["out"]
    return out



# revision 18
# speedup vs baseline: 1.6610x; 1.6610x over previous
"""DeformConv3D on 8 TRN2 cores, single fused launch.

Structure (fully SPMD-symmetric; per-core asymmetry only via host-sliced
inputs and the two AllToAll collectives):
  P1 (h-sharded): offset conv rows [12k,12k+12), all 192 ch, bf16 matmul,
      written f32 into cc1_src grouped by destination core's channel block.
  A2A#1 + reshuffle: re-shard offsets to plane-groups; assemble e_loc =
      per-plane channel-major flat field so the torch contiguous-view
      scramble becomes an affine strided read.
  P2 (plane-sharded): 16 planes x 8 h-chunks = 128 partitions; 5-tap
      separable tent-weight trilinear gather. Tents on ACT engine, bf16
      DVE inner loop, x-shifted window copy keeps 2x DVE mode on odd taps.
  A2A#2: re-shard deformed field back to h-bands.
  P3: main conv + bias (bf16 matmul, bias on ACT).
"""
import sys
import numpy as np

sys.path.insert(0, "/opt/trn_rl_repo")
from concourse import bass, bacc, tile, mybir
from concourse.bass_utils import run_bass_kernel_spmd

F32 = mybir.dt.float32
BF16 = mybir.dt.bfloat16
ALU = mybir.AluOpType
AF = mybir.ActivationFunctionType

B, C, L, H, W = 2, 64, 16, 96, 96
CO1, CO2 = 192, 64
NCORES = 8
HB = 12            # output rows per core (h band)
PL = 16            # planes per core in phase 2
TAPS = (-2, -1, 0, 1, 2)
LHW = L * H * W    # 147456
U = 3 * W          # 288 scrambled elems per (l, h) row

_nc_cache = None


def build_program():
    nc = bacc.Bacc("TRN2", target_bir_lowering=False, debug=False,
                   num_devices=NCORES)
    xf = nc.dram_tensor("xf", [C, L, 98, 98], BF16,
                        kind="ExternalInput").ap()
    xg = nc.dram_tensor("xg", [PL, L + 4, 100, 100], BF16,
                        kind="ExternalInput").ap()
    xg1 = nc.dram_tensor("xg1", [PL, L + 4, 100, 100], BF16,
                         kind="ExternalInput").ap()
    w_off_t = nc.dram_tensor("w_off_t", [C, 27, 48], BF16,
                             kind="ExternalInput").ap()
    w_conv_t = nc.dram_tensor("w_conv_t", [C, 27, CO2], BF16,
                              kind="ExternalInput").ap()
    b_conv = nc.dram_tensor("b_conv", [CO2, 1], F32,
                            kind="ExternalInput").ap()
    out_ext = nc.dram_tensor("out", [B, CO2, L, HB, W], F32,
                             kind="ExternalOutput").ap()

    e_loc = nc.dram_tensor("e_loc", [PL, 3 * LHW], F32).ap()
    def_loc = nc.dram_tensor("def_loc", [PL, L, 98, W], BF16).ap()
    cc2_src = nc.dram_tensor("cc2_src", [NCORES, PL, L, 14, W], BF16).ap()
    cc2_dst = nc.dram_tensor("cc2_dst", [NCORES, PL, L, 14, W], BF16).ap()

    # grid constants: partition p = j*16 + pl, free = (i<12, w<96)
    pp = np.arange(128)
    gy_np = np.broadcast_to(
        (12.0 * (pp // 16))[:, None, None]
        + np.arange(HB, dtype=np.float32)[None, :, None],
        (128, HB, W)).reshape(128, HB * W).astype(np.float32).copy()
    gx_np = np.broadcast_to(
        np.arange(W, dtype=np.float32)[None, None, :],
        (128, HB, W)).reshape(128, HB * W).astype(np.float32).copy()
    gy_d = nc.inline_tensor(gy_np, name="gy_const").ap()
    gx_d = nc.inline_tensor(gx_np, name="gx_const").ap()

    NPP = HB * W  # 1152

    with tile.TileContext(nc) as tc:
        # ------- Phase 1: offset conv, this core's 48 channels, full H -------
        # writes conv output directly into e_loc (channel-major per plane)
        e_w = e_loc.rearrange("pl (c3 l h w) -> (pl c3) l h w",
                              c3=3, l=L, h=H, w=W)
        HCH = tuple((5 * i, 5) for i in range(19)) + ((95, 1),)
        with tc.tile_pool(name="p1", bufs=1) as p1, \
             tc.tile_pool(name="p1x", bufs=4) as p1x, \
             tc.tile_pool(name="p1ps", bufs=6, space="PSUM") as p1ps, \
             tc.tile_pool(name="p1o", bufs=4) as p1o:
            wofft = p1.tile([C, 27, 48], BF16)
            nc.sync.dma_start(wofft[:], w_off_t)
            zslice = p1.tile([C, 98, 98], BF16, tag="zz")
            nc.vector.memset(zslice[:].rearrange("c y x -> c (y x)"), 0.0)
            zs = []
            for l in range(L):
                t = p1x.tile([C, 98, 98], BF16, tag="xz")
                nc.sync.dma_start(t[:], xf[:, l])
                zs.append(t)
            for l in range(L):
                tz = [zs[l - 1] if l > 0 else zslice,
                      zs[l],
                      zs[l + 1] if l < L - 1 else zslice]
                for hc0, hcn in HCH:
                    nmm = hcn * W
                    ps = p1ps.tile([48, 480], F32, tag="ps1")
                    for t in range(27):
                        dz, rem = divmod(t, 9)
                        dy, dx = divmod(rem, 3)
                        rhs = tz[dz][:, hc0 + dy:hc0 + dy + hcn,
                                     dx:dx + W]
                        nc.tensor.matmul(
                            ps[:, :nmm], wofft[:, t, :],
                            rhs, start=(t == 0), stop=(t == 26))
                    ob = p1o.tile([48, 480], F32, tag="ob1")
                    nc.scalar.copy(ob[:, :nmm], ps[:, :nmm])
                    nc.sync.dma_start(
                        e_w[:, l, hc0:hc0 + hcn, :],
                        ob[:, :nmm].rearrange("p (r w) -> p r w", w=W))

        # ---------------- Phase 2: tent gather (plane-sharded) ----------
        # partition p = j*16 + pl handles plane pl rows [12j, 12j+12)
        e_v = e_loc.rearrange("pl (l j i u) -> l j pl (i u)",
                              l=L, j=NCORES, i=HB, u=U)
        with tc.tile_pool(name="p2c", bufs=1) as p2c, \
             tc.tile_pool(name="p2w", bufs=2) as p2w, \
             tc.tile_pool(name="p2r", bufs=2) as p2r, \
             tc.tile_pool(name="p2a2", bufs=2) as p2a2, \
             tc.tile_pool(name="p2a", bufs=1) as p2a, \
             tc.tile_pool(name="p2l", bufs=2) as p2l, \
             tc.tile_pool(name="p2t", bufs=1) as p2t, \
             tc.tile_pool(name="p2acc", bufs=2) as p2acc:
            gyt = p2c.tile([128, NPP], F32)
            nc.sync.dma_start(gyt[:], gy_d)
            gxt = p2c.tile([128, NPP], F32)
            nc.sync.dma_start(gxt[:], gx_d)
            negt = {}
            for t in TAPS:
                cb = p2c.tile([128, 1], F32, tag=f"negt{t}")
                nc.vector.memset(cb[:], -float(t))
                negt[t] = cb
            for l in range(L):
                win_e = p2w.tile([128, 5, 16, 100], BF16, tag="we")
                for j in range(NCORES):
                    nc.sync.dma_start(
                        win_e[16 * j:16 * j + 16],
                        xg[:, l:l + 5, 12 * j:12 * j + 16, :])
                win_o = p2w.tile([128, 5, 16, 100], BF16, tag="wo")
                for j in range(NCORES):
                    nc.sync.dma_start(
                        win_o[16 * j:16 * j + 16],
                        xg1[:, l:l + 5, 12 * j:12 * j + 16, :])

                raw = p2r.tile([128, HB * U], F32, tag="raw")
                for j in range(NCORES):
                    nc.sync.dma_start(raw[16 * j:16 * j + 16], e_v[l, j])
                raw = raw.rearrange("p (i u) -> p i u", u=U)
                rv = raw.rearrange("p i (w k) -> p k (i w)", k=3)

                az = p2a2.tile([128, NPP], F32, tag="az")
                nc.vector.tensor_scalar(az[:], rv[:, 0], float(l), 0.0,
                                        ALU.add, ALU.max)
                nc.vector.tensor_scalar(az[:], az[:], 15.0, -float(l),
                                        ALU.min, ALU.add)
                ay = p2a.tile([128, NPP], F32, tag="ay")
                nc.vector.tensor_tensor(ay[:], rv[:, 1], gyt[:], ALU.add)
                nc.vector.tensor_scalar(ay[:], ay[:], 0.0, 95.0,
                                        ALU.max, ALU.min)
                nc.vector.tensor_tensor(ay[:], ay[:], gyt[:], ALU.subtract)
                ax = p2a.tile([128, NPP], F32, tag="ax")
                nc.vector.tensor_tensor(ax[:], rv[:, 2], gxt[:], ALU.add)
                nc.vector.tensor_scalar(ax[:], ax[:], 0.0, 95.0,
                                        ALU.max, ALU.min)
                nc.vector.tensor_tensor(ax[:], ax[:], gxt[:], ALU.subtract)

                def tents(a, nm):
                    row = []
                    for idx, t in enumerate(TAPS):
                        ta = p2a.tile([128, NPP], F32, tag="tact")
                        nc.scalar.activation(ta[:], a[:], AF.Abs,
                                             bias=negt[t][:])
                        lt = p2l.tile([128, NPP], BF16, tag=f"{nm}{idx}")
                        nc.scalar.activation(lt[:], ta[:], AF.Relu,
                                             bias=1.0, scale=-1.0)
                        row.append(lt)
                    return row

                lamx = tents(ax, "lx")
                lamy = tents(ay, "ly")

                acc = p2acc.tile([128, NPP], BF16, tag="acc")
                tmpb = p2t.tile([128, NPP], BF16, tag="tmpb")
                tmpi = p2t.tile([128, NPP], BF16, tag="tmpi")
                prod = p2t.tile([128, NPP], BF16, tag="prod")
                tmpi_v = tmpi.rearrange("p (i w) -> p i w", w=W)
                for iz, sz in enumerate(TAPS):
                    ta2 = p2a.tile([128, NPP], F32, tag="tact2")
                    nc.scalar.activation(ta2[:], az[:], AF.Abs,
                                         bias=negt[sz][:])
                    lz = p2l.tile([128, NPP], BF16, tag="lz")
                    nc.scalar.activation(lz[:], ta2[:], AF.Relu,
                                         bias=1.0, scale=-1.0)
                    for iy, sy in enumerate(TAPS):
                        for ix, sx in enumerate(TAPS):
                            if sx % 2 == 0:
                                v = win_e[:, 2 + sz, 2 + sy:14 + sy,
                                          2 + sx:98 + sx]
                            else:
                                v = win_o[:, 2 + sz, 2 + sy:14 + sy,
                                          1 + sx:97 + sx]
                            lxv = lamx[ix].rearrange("p (i w) -> p i w", w=W)
                            if ix == 0:
                                nc.vector.tensor_tensor(tmpi_v[:], lxv, v,
                                                        ALU.mult)
                            else:
                                pv = prod.rearrange("p (i w) -> p i w", w=W)
                                nc.vector.tensor_tensor(pv[:], lxv, v,
                                                        ALU.mult)
                                nc.vector.tensor_tensor(tmpi[:], tmpi[:],
                                                        prod[:], ALU.add)
                        if iy == 0:
                            nc.vector.tensor_tensor(tmpb[:], lamy[0][:],
                                                    tmpi[:], ALU.mult)
                        else:
                            nc.vector.tensor_tensor(tmpi[:], lamy[iy][:],
                                                    tmpi[:], ALU.mult)
                            nc.vector.tensor_tensor(tmpb[:], tmpb[:],
                                                    tmpi[:], ALU.add)
                    if iz == 0:
                        nc.vector.tensor_tensor(acc[:], lz[:], tmpb[:],
                                                ALU.mult)
                    else:
                        nc.vector.tensor_tensor(tmpb[:], lz[:], tmpb[:],
                                                ALU.mult)
                        nc.vector.tensor_tensor(acc[:], acc[:], tmpb[:],
                                                ALU.add)
                for j in range(NCORES):
                    nc.sync.dma_start(
                        def_loc[:, l, 1 + 12 * j:13 + 12 * j, :],
                        acc[16 * j:16 * j + 16]
                        .rearrange("pl (i w) -> pl i w", w=W))

            # zero def_loc boundary rows (g=0 and g=97)
            zt = p2c.tile([PL, L, W], BF16, tag="zt")
            nc.vector.memset(zt[:].rearrange("p l w -> p (l w)"), 0.0)
            for g in (0, 97):
                nc.sync.dma_start(def_loc[:, :, g, :], zt[:])

        # ---------------- A2A #2: deformed to h-bands ----------------
        for d in range(NCORES):
            nc.sync.dma_start(cc2_src[d],
                              def_loc[:, :, HB * d:HB * d + 14, :])
        nc.gpsimd.collective_compute(
            "AllToAll", ALU.bypass,
            replica_groups=[list(range(NCORES))],
            ins=[cc2_src], outs=[cc2_dst])

        # ---------------- Phase 3: main conv ----------------
        with tc.tile_pool(name="p3", bufs=1) as p3, \
             tc.tile_pool(name="p3ps", bufs=6, space="PSUM") as p3ps, \
             tc.tile_pool(name="p3o", bufs=4) as p3o:
            wct = p3.tile([C, 27, CO2], BF16)
            nc.sync.dma_start(wct[:], w_conv_t)
            bct = p3.tile([CO2, 1], F32)
            nc.sync.dma_start(bct[:], b_conv)
            for b in range(B):
                dc = p3.tile([C, L + 2, 14, 98], BF16, tag=f"dc{b}")
                # zero pad: z slices 0,17 and x cols 0,97
                nc.vector.memset(dc[:, 0], 0.0)
                nc.vector.memset(dc[:, L + 1], 0.0)
                nc.vector.memset(dc[:, :, :, 0], 0.0)
                nc.vector.memset(dc[:, :, :, 97], 0.0)
                cd = cc2_dst[4 * b:4 * b + 4].rearrange(
                    "jj pl l i w -> (jj pl) l i w")
                for i in range(14):
                    nc.sync.dma_start(dc[:, 1:L + 1, i, 1:97], cd[:, :, i])
                for l in range(L):
                    for hc0, hcn in ((0, 5), (5, 5), (10, 2)):
                        nmm = hcn * W
                        ps = p3ps.tile([CO2, 480], F32, tag="ps3")
                        for t in range(27):
                            dz, rem = divmod(t, 9)
                            dy, dx = divmod(rem, 3)
                            rhs = dc[:, l + dz,
                                     hc0 + dy:hc0 + dy + hcn,
                                     dx:dx + W]
                            nc.tensor.matmul(
                                ps[:, :nmm], wct[:, t, :],
                                rhs, start=(t == 0), stop=(t == 26))
                        ob = p3o.tile([CO2, 480], F32, tag="ob3")
                        nc.scalar.activation(ob[:, :nmm], ps[:, :nmm],
                                             AF.Identity, bias=bct[:])
                        nc.sync.dma_start(
                            out_ext[b, :, l, hc0:hc0 + hcn, :]
                            .rearrange("m h x -> m (h x)"),
                            ob[:, :nmm])
    nc.finalize()
    return nc


def kernel(x, w_off, w_conv, b_conv):
    global _nc_cache
    import ml_dtypes
    x = np.asarray(x, dtype=np.float32)
    w_off = np.asarray(w_off, dtype=np.float32)
    w_conv = np.asarray(w_conv, dtype=np.float32)
    b_conv = np.asarray(b_conv, dtype=np.float32)

    if _nc_cache is None:
        _nc_cache = build_program()

    bf = ml_dtypes.bfloat16
    # phase-1 input: full-H slab of this core's batch, y/x pad +-1
    xfull = np.zeros((B, C, L, 98, 98), dtype=bf)
    xfull[:, :, :, 1:H + 1, 1:W + 1] = x
    # phase-2 input: plane-padded by 2 in z/y/x
    x4 = x.reshape(B * C, L, H, W)
    xq = np.zeros((B * C, L + 4, 100, 100), dtype=bf)
    xq[:, 2:L + 2, 2:H + 2, 2:W + 2] = x4
    xq1 = np.zeros((B * C, L + 4, 100, 100), dtype=bf)
    xq1[:, 2:L + 2, 2:H + 2, 1:W + 1] = x4

    wofft = np.ascontiguousarray(
        w_off.reshape(CO1, C, 27).transpose(1, 2, 0)).astype(bf)
    wct = np.ascontiguousarray(
        w_conv.reshape(CO2, C, 27).transpose(1, 2, 0)).astype(bf)
    bc = np.ascontiguousarray(b_conv.reshape(CO2, 1))

    in_maps = []
    for k in range(NCORES):
        c4 = k % 4
        in_maps.append({
            "xf": np.ascontiguousarray(xfull[k // 4]),
            "xg": np.ascontiguousarray(xq[PL * k:PL * k + PL]),
            "xg1": np.ascontiguousarray(xq1[PL * k:PL * k + PL]),
            "w_off_t": np.ascontiguousarray(
                wofft[:, :, 48 * c4:48 * c4 + 48]),
            "w_conv_t": wct,
            "b_conv": bc,
        })
    res = run_bass_kernel_spmd(_nc_cache, in_maps, list(range(NCORES)))
    out = np.empty((B, CO2, L, H, W), np.float32)
    for k in range(NCORES):
        out[:, :, :, 12 * k:12 * k + HB, :] = res.results[k]["out"]
    return out


# revision 20
# speedup vs baseline: 1.7315x; 1.0425x over previous
"""DeformConv3D on 8 TRN2 cores, single fused launch.

Structure (fully SPMD-symmetric; per-core asymmetry only via host-sliced
inputs and the two AllToAll collectives):
  P1 (h-sharded): offset conv rows [12k,12k+12), all 192 ch, bf16 matmul,
      written f32 into cc1_src grouped by destination core's channel block.
  A2A#1 + reshuffle: re-shard offsets to plane-groups; assemble e_loc =
      per-plane channel-major flat field so the torch contiguous-view
      scramble becomes an affine strided read.
  P2 (plane-sharded): 16 planes x 8 h-chunks = 128 partitions; 5-tap
      separable tent-weight trilinear gather. Tents on ACT engine, bf16
      DVE inner loop, x-shifted window copy keeps 2x DVE mode on odd taps.
  A2A#2: re-shard deformed field back to h-bands.
  P3: main conv + bias (bf16 matmul, bias on ACT).
"""
import sys
import numpy as np

sys.path.insert(0, "/opt/trn_rl_repo")
from concourse import bass, bacc, tile, mybir
from concourse.bass_utils import run_bass_kernel_spmd

F32 = mybir.dt.float32
BF16 = mybir.dt.bfloat16
ALU = mybir.AluOpType
AF = mybir.ActivationFunctionType

B, C, L, H, W = 2, 64, 16, 96, 96
CO1, CO2 = 192, 64
NCORES = 8
HB = 12            # output rows per core (h band)
PL = 16            # planes per core in phase 2
TAPS = (-2, -1, 0, 1, 2)
LHW = L * H * W    # 147456
U = 3 * W          # 288 scrambled elems per (l, h) row

_nc_cache = None


def build_program():
    nc = bacc.Bacc("TRN2", target_bir_lowering=False, debug=False,
                   num_devices=NCORES)
    xf = nc.dram_tensor("xf", [C, L + 4, 100, 100], BF16,
                        kind="ExternalInput").ap()
    w_off_t = nc.dram_tensor("w_off_t", [C, 27, 48], BF16,
                             kind="ExternalInput").ap()
    w_conv_t = nc.dram_tensor("w_conv_t", [C, 27, CO2], BF16,
                              kind="ExternalInput").ap()
    b_conv = nc.dram_tensor("b_conv", [CO2, 1], F32,
                            kind="ExternalInput").ap()
    out_ext = nc.dram_tensor("out", [B, CO2, L, HB, W], BF16,
                             kind="ExternalOutput").ap()

    e_loc = nc.dram_tensor("e_loc", [PL, 3 * LHW], F32).ap()
    def_loc = nc.dram_tensor("def_loc", [PL, L, 98, W], BF16).ap()
    cc2_src = nc.dram_tensor("cc2_src", [NCORES, PL, L, 14, W], BF16).ap()
    cc2_dst = nc.dram_tensor("cc2_dst", [NCORES, PL, L, 14, W], BF16).ap()

    # grid constants: partition p = j*16 + pl, free = (i<12, w<96)
    pp = np.arange(128)
    gy_np = np.broadcast_to(
        (12.0 * (pp // 16))[:, None, None]
        + np.arange(HB, dtype=np.float32)[None, :, None],
        (128, HB, W)).reshape(128, HB * W).astype(np.float32).copy()
    gx_np = np.broadcast_to(
        np.arange(W, dtype=np.float32)[None, None, :],
        (128, HB, W)).reshape(128, HB * W).astype(np.float32).copy()
    gy_d = nc.inline_tensor(gy_np, name="gy_const").ap()
    gx_d = nc.inline_tensor(gx_np, name="gx_const").ap()

    NPP = HB * W  # 1152

    with tile.TileContext(nc) as tc:
        # ------- Phase 1: offset conv, this core's 48 channels, full H -------
        # writes conv output directly into e_loc (channel-major per plane)
        e_w = e_loc.rearrange("pl (c3 l h w) -> (pl c3) l h w",
                              c3=3, l=L, h=H, w=W)
        HCH = tuple((5 * i, 5) for i in range(19)) + ((95, 1),)
        with tc.tile_pool(name="p1", bufs=1) as p1, \
             tc.tile_pool(name="p1x", bufs=4) as p1x, \
             tc.tile_pool(name="p1ps", bufs=6, space="PSUM") as p1ps, \
             tc.tile_pool(name="p1o", bufs=4) as p1o:
            wofft = p1.tile([C, 27, 48], BF16)
            nc.sync.dma_start(wofft[:], w_off_t)
            zs = {}
            for zi in range(1, L + 3):
                t = p1x.tile([C, 100, 100], BF16, tag="xz")
                nc.sync.dma_start(t[:], xf[:, zi])
                zs[zi] = t
            for l in range(L):
                for hc0, hcn in HCH:
                    nmm = hcn * W
                    ps = p1ps.tile([48, 480], F32, tag="ps1")
                    for t in range(27):
                        dz, rem = divmod(t, 9)
                        dy, dx = divmod(rem, 3)
                        rhs = zs[l + dz + 1][:, hc0 + dy + 1:
                                             hc0 + dy + 1 + hcn,
                                             dx + 1:dx + 1 + W]
                        nc.tensor.matmul(
                            ps[:, :nmm], wofft[:, t, :],
                            rhs, start=(t == 0), stop=(t == 26))
                    ob = p1o.tile([48, 480], F32, tag="ob1")
                    nc.scalar.copy(ob[:, :nmm], ps[:, :nmm])
                    nc.sync.dma_start(
                        e_w[:, l, hc0:hc0 + hcn, :],
                        ob[:, :nmm].rearrange("p (r w) -> p r w", w=W))

        # ---------------- Phase 2: tent gather (plane-sharded) ----------
        # partition p = j*16 + pl handles plane pl rows [12j, 12j+12)
        e_v = e_loc.rearrange("pl (l j i u) -> l j pl (i u)",
                              l=L, j=NCORES, i=HB, u=U)
        with tc.tile_pool(name="p2c", bufs=1) as p2c, \
             tc.tile_pool(name="p2w", bufs=2) as p2w, \
             tc.tile_pool(name="p2r", bufs=2) as p2r, \
             tc.tile_pool(name="p2a2", bufs=2) as p2a2, \
             tc.tile_pool(name="p2a", bufs=1) as p2a, \
             tc.tile_pool(name="p2l", bufs=2) as p2l, \
             tc.tile_pool(name="p2t", bufs=1) as p2t, \
             tc.tile_pool(name="p2acc", bufs=2) as p2acc:
            gyt = p2c.tile([128, NPP], F32)
            nc.sync.dma_start(gyt[:], gy_d)
            gxt = p2c.tile([128, NPP], F32)
            nc.sync.dma_start(gxt[:], gx_d)
            negt = {}
            for t in TAPS:
                cb = p2c.tile([128, 1], F32, tag=f"negt{t}")
                nc.vector.memset(cb[:], -float(t))
                negt[t] = cb
            for l in range(L):
                # win_e[p=(j,pl), z5, y, x] = x4[pl, l+z5-2, 12j+y-2, x-2]
                # from xf (y/x pad 1): xf[pl, l+z5-2, 12j+y-1, x-1]
                win_e = p2w.tile([128, 5, 16, 100], BF16, tag="we")
                for j in range(NCORES):
                    nc.sync.dma_start(
                        win_e[16 * j:16 * j + 16],
                        xf[0:PL, l:l + 5, 12 * j:12 * j + 16, :])
                win_o = p2w.tile([128, 5, 16, 100], BF16, tag="wo")
                nc.gpsimd.tensor_copy(
                    win_o[:, :, :, 0:99], win_e[:, :, :, 1:100])

                raw = p2r.tile([128, HB * U], F32, tag="raw")
                for j in range(NCORES):
                    nc.sync.dma_start(raw[16 * j:16 * j + 16], e_v[l, j])
                raw = raw.rearrange("p (i u) -> p i u", u=U)
                rv = raw.rearrange("p i (w k) -> p k (i w)", k=3)

                az = p2a2.tile([128, NPP], F32, tag="az")
                nc.vector.tensor_scalar(az[:], rv[:, 0], float(l), 0.0,
                                        ALU.add, ALU.max)
                nc.vector.tensor_scalar(az[:], az[:], 15.0, -float(l),
                                        ALU.min, ALU.add)
                ay = p2a.tile([128, NPP], F32, tag="ay")
                nc.vector.tensor_tensor(ay[:], rv[:, 1], gyt[:], ALU.add)
                nc.vector.tensor_scalar(ay[:], ay[:], 0.0, 95.0,
                                        ALU.max, ALU.min)
                nc.vector.tensor_tensor(ay[:], ay[:], gyt[:], ALU.subtract)
                ax = p2a.tile([128, NPP], F32, tag="ax")
                nc.vector.tensor_tensor(ax[:], rv[:, 2], gxt[:], ALU.add)
                nc.vector.tensor_scalar(ax[:], ax[:], 0.0, 95.0,
                                        ALU.max, ALU.min)
                nc.vector.tensor_tensor(ax[:], ax[:], gxt[:], ALU.subtract)

                def tents(a, nm):
                    row = []
                    for idx, t in enumerate(TAPS):
                        ta = p2a.tile([128, NPP], F32, tag="tact")
                        nc.scalar.activation(ta[:], a[:], AF.Abs,
                                             bias=negt[t][:])
                        lt = p2l.tile([128, NPP], BF16, tag=f"{nm}{idx}")
                        nc.scalar.activation(lt[:], ta[:], AF.Relu,
                                             bias=1.0, scale=-1.0)
                        row.append(lt)
                    return row

                lamx = tents(ax, "lx")
                lamy = tents(ay, "ly")

                acc = p2acc.tile([128, NPP], BF16, tag="acc")
                tmpb = p2t.tile([128, NPP], BF16, tag="tmpb")
                tmpi = p2t.tile([128, NPP], BF16, tag="tmpi")
                prod = p2t.tile([128, NPP], BF16, tag="prod")
                tmpi_v = tmpi.rearrange("p (i w) -> p i w", w=W)
                for iz, sz in enumerate(TAPS):
                    ta2 = p2a.tile([128, NPP], F32, tag="tact2")
                    nc.scalar.activation(ta2[:], az[:], AF.Abs,
                                         bias=negt[sz][:])
                    lz = p2l.tile([128, NPP], BF16, tag="lz")
                    nc.scalar.activation(lz[:], ta2[:], AF.Relu,
                                         bias=1.0, scale=-1.0)
                    for iy, sy in enumerate(TAPS):
                        for ix, sx in enumerate(TAPS):
                            if sx % 2 == 0:
                                v = win_e[:, 2 + sz, 2 + sy:14 + sy,
                                          2 + sx:98 + sx]
                            else:
                                v = win_o[:, 2 + sz, 2 + sy:14 + sy,
                                          1 + sx:97 + sx]
                            lxv = lamx[ix].rearrange("p (i w) -> p i w", w=W)
                            if ix == 0:
                                nc.vector.tensor_tensor(tmpi_v[:], lxv, v,
                                                        ALU.mult)
                            else:
                                pv = prod.rearrange("p (i w) -> p i w", w=W)
                                nc.vector.tensor_tensor(pv[:], lxv, v,
                                                        ALU.mult)
                                nc.vector.tensor_tensor(tmpi[:], tmpi[:],
                                                        prod[:], ALU.add)
                        if iy == 0:
                            nc.vector.tensor_tensor(tmpb[:], lamy[0][:],
                                                    tmpi[:], ALU.mult)
                        else:
                            nc.vector.tensor_tensor(tmpi[:], lamy[iy][:],
                                                    tmpi[:], ALU.mult)
                            nc.vector.tensor_tensor(tmpb[:], tmpb[:],
                                                    tmpi[:], ALU.add)
                    if iz == 0:
                        nc.vector.tensor_tensor(acc[:], lz[:], tmpb[:],
                                                ALU.mult)
                    else:
                        nc.vector.tensor_tensor(tmpb[:], lz[:], tmpb[:],
                                                ALU.mult)
                        nc.vector.tensor_tensor(acc[:], acc[:], tmpb[:],
                                                ALU.add)
                for j in range(NCORES):
                    nc.sync.dma_start(
                        def_loc[:, l, 1 + 12 * j:13 + 12 * j, :],
                        acc[16 * j:16 * j + 16]
                        .rearrange("pl (i w) -> pl i w", w=W))

            # zero def_loc boundary rows (g=0 and g=97)
            zt = p2c.tile([PL, L, W], BF16, tag="zt")
            nc.vector.memset(zt[:].rearrange("p l w -> p (l w)"), 0.0)
            for g in (0, 97):
                nc.sync.dma_start(def_loc[:, :, g, :], zt[:])

        # ---------------- A2A #2: deformed to h-bands ----------------
        for d in range(NCORES):
            nc.sync.dma_start(cc2_src[d],
                              def_loc[:, :, HB * d:HB * d + 14, :])
        nc.gpsimd.collective_compute(
            "AllToAll", ALU.bypass,
            replica_groups=[list(range(NCORES))],
            ins=[cc2_src], outs=[cc2_dst])

        # ---------------- Phase 3: main conv ----------------
        with tc.tile_pool(name="p3", bufs=1) as p3, \
             tc.tile_pool(name="p3ps", bufs=6, space="PSUM") as p3ps, \
             tc.tile_pool(name="p3o", bufs=4) as p3o:
            wct = p3.tile([C, 27, CO2], BF16)
            nc.sync.dma_start(wct[:], w_conv_t)
            bct = p3.tile([CO2, 1], F32)
            nc.sync.dma_start(bct[:], b_conv)
            for b in range(B):
                dc = p3.tile([C, L + 2, 14, 98], BF16, tag=f"dc{b}")
                # zero pad: z slices 0,17 and x cols 0,97
                nc.vector.memset(dc[:, 0], 0.0)
                nc.vector.memset(dc[:, L + 1], 0.0)
                nc.vector.memset(dc[:, :, :, 0], 0.0)
                nc.vector.memset(dc[:, :, :, 97], 0.0)
                cd = cc2_dst[4 * b:4 * b + 4].rearrange(
                    "jj pl l i w -> (jj pl) l i w")
                for i in range(14):
                    nc.sync.dma_start(dc[:, 1:L + 1, i, 1:97], cd[:, :, i])
                for l in range(L):
                    for hc0, hcn in ((0, 5), (5, 5), (10, 2)):
                        nmm = hcn * W
                        ps = p3ps.tile([CO2, 480], F32, tag="ps3")
                        for t in range(27):
                            dz, rem = divmod(t, 9)
                            dy, dx = divmod(rem, 3)
                            rhs = dc[:, l + dz,
                                     hc0 + dy:hc0 + dy + hcn,
                                     dx:dx + W]
                            nc.tensor.matmul(
                                ps[:, :nmm], wct[:, t, :],
                                rhs, start=(t == 0), stop=(t == 26))
                        ob = p3o.tile([CO2, 480], BF16, tag="ob3")
                        nc.scalar.activation(ob[:, :nmm], ps[:, :nmm],
                                             AF.Identity, bias=bct[:])
                        nc.sync.dma_start(
                            out_ext[b, :, l, hc0:hc0 + hcn, :]
                            .rearrange("m h x -> m (h x)"),
                            ob[:, :nmm])
    nc.finalize()
    return nc


def kernel(x, w_off, w_conv, b_conv):
    global _nc_cache
    import ml_dtypes
    x = np.asarray(x, dtype=np.float32)
    w_off = np.asarray(w_off, dtype=np.float32)
    w_conv = np.asarray(w_conv, dtype=np.float32)
    b_conv = np.asarray(b_conv, dtype=np.float32)

    if _nc_cache is None:
        _nc_cache = build_program()

    bf = ml_dtypes.bfloat16
    # phase-1 input: full-H slab of this core's batch, y/x pad +-1.
    # channels permuted per core so the core's 16 gather planes are the
    # first 16 channels (contraction is order-invariant; w_off_t rows are
    # permuted identically).
    xfull = np.zeros((B, C, L + 4, 100, 100), dtype=bf)
    xfull[:, :, 2:L + 2, 2:H + 2, 2:W + 2] = x

    wofft = np.ascontiguousarray(
        w_off.reshape(CO1, C, 27).transpose(1, 2, 0)).astype(bf)
    wct = np.ascontiguousarray(
        w_conv.reshape(CO2, C, 27).transpose(1, 2, 0)).astype(bf)
    bc = np.ascontiguousarray(b_conv.reshape(CO2, 1))

    in_maps = []
    for k in range(NCORES):
        c4 = k % 4
        perm = np.concatenate([np.arange(16 * c4, 16 * c4 + 16),
                               np.arange(0, 16 * c4),
                               np.arange(16 * c4 + 16, C)])
        in_maps.append({
            "xf": np.ascontiguousarray(xfull[k // 4][perm]),
            "w_off_t": np.ascontiguousarray(
                wofft[perm][:, :, 48 * c4:48 * c4 + 48]),
            "w_conv_t": wct,
            "b_conv": bc,
        })
    res = run_bass_kernel_spmd(_nc_cache, in_maps, list(range(NCORES)))
    out = np.empty((B, CO2, L, H, W), np.float32)
    for k in range(NCORES):
        out[:, :, :, 12 * k:12 * k + HB, :] = \
            res.results[k]["out"].astype(np.float32)
    return out
